# revision 1
# baseline (speedup 1.0000x reference)
"""Local causal (sliding-window) attention block on 8 TRN2 NeuronCores.

Reference computation (per batch b):
    h = LayerNorm(x) * gamma + beta
    Q = h@Wq, K = h@Wk, V = h@Wv          (heads: 16 x 64)
    S = QK^T/sqrt(dk) masked to causal band of width 256
    out = x + softmax(S)@V @ Wo + bo

Sharding: 8 cores = 2 batches x 4 head-groups (4 heads each).
Each core computes LN(x_b), its head-group's Q/K/V, banded attention,
and a partial out-projection  attn_g @ Wo[g]  (token-major, [T, D]).
Host reduces: out[b] = x[b] + sum_g partial[b,g] + bo.

gamma (and 1/sqrt(dk) for Q) are folded into the projection weights on
the host; beta enters via folded bias rows beta@W.
"""

import os

import numpy as np

import concourse.bass as bass
import concourse.tile as tile
from concourse import bacc, mybir
from concourse.bass_utils import run_bass_kernel_spmd

F32 = mybir.dt.float32
F32R = mybir.dt.float32r
BF16 = mybir.dt.bfloat16

T = 2048          # tokens per batch
D = 1024          # model dim
HG = 4            # heads per core
DK = 64           # head dim
DG = HG * DK      # head-group feature width (256)
WIN = 256         # attention window
P = 128           # partitions
NT = T // P       # 16 token tiles
KC = D // P       # 8 feature chunks
LN_EPS = 1e-5
MASKVAL = -1e9

# filled by test.py via run(trace=True)
LAST_PROFILE = {}


def _rc(ap):
    return ap


def _body(tc):
    nc = tc.nc

    x = nc.dram_tensor("x", [T, D], F32, kind="ExternalInput").ap()
    wq = nc.dram_tensor("wq", [D, DG], F32R, kind="ExternalInput").ap()
    wk = nc.dram_tensor("wk", [D, DG], F32R, kind="ExternalInput").ap()
    wv = nc.dram_tensor("wv", [D, DG], F32R, kind="ExternalInput").ap()
    wo = nc.dram_tensor("wo", [DG, D], F32R, kind="ExternalInput").ap()
    bq = nc.dram_tensor("bq", [P, DG // P], F32, kind="ExternalInput").ap()
    bk = nc.dram_tensor("bk", [P, DG // P], F32, kind="ExternalInput").ap()
    bv = nc.dram_tensor("bv", [P, DG], F32, kind="ExternalInput").ap()
    mup = nc.dram_tensor("mup", [P, P], F32, kind="ExternalInput").ap()
    mlo = nc.dram_tensor("mlo", [P, P], F32, kind="ExternalInput").ap()
    idf = nc.dram_tensor("idf", [P, P], F32R, kind="ExternalInput").ap()
    idb = nc.dram_tensor("idb", [P, P], BF16, kind="ExternalInput").ap()
    partial = nc.dram_tensor("partial", [T, D], F32, kind="ExternalOutput").ap()

    with (
        tc.tile_pool(name="consts", bufs=1) as consts,
        tc.tile_pool(name="big", bufs=1) as big,
    ):
        # ---- resident SBUF tensors ----
        wq_sb = consts.tile([P, KC, DG], F32R, tag="wq")
        wk_sb = consts.tile([P, KC, DG], F32R, tag="wk")
        wv_sb = consts.tile([P, KC, DG], F32R, tag="wv")
        wo_sb = consts.tile([P, DG // P, D], F32R, tag="wo")
        bq_sb = consts.tile([P, DG // P], F32, tag="bq")
        bk_sb = consts.tile([P, DG // P], F32, tag="bk")
        bv_sb = consts.tile([P, DG], F32, tag="bv")
        mup_sb = consts.tile([P, P], F32, tag="mup")
        mlo_sb = consts.tile([P, P], F32, tag="mlo")
        idf_sb = consts.tile([P, P], F32R, tag="idf")
        idb_sb = consts.tile([P, P], BF16, tag="idb")
        eps_sb = consts.tile([P, 1], F32, tag="eps")

        nc.sync.dma_start(out=wq_sb, in_=wq.rearrange("(c p) n -> p c n", p=P))
        nc.sync.dma_start(out=wk_sb, in_=wk.rearrange("(c p) n -> p c n", p=P))
        nc.sync.dma_start(out=wv_sb, in_=wv.rearrange("(c p) n -> p c n", p=P))
        nc.sync.dma_start(out=wo_sb, in_=wo.rearrange("(c p) n -> p c n", p=P))
        nc.sync.dma_start(out=bq_sb, in_=bq)
        nc.sync.dma_start(out=bk_sb, in_=bk)
        nc.sync.dma_start(out=bv_sb, in_=bv)
        nc.sync.dma_start(out=mup_sb, in_=mup)
        nc.sync.dma_start(out=mlo_sb, in_=mlo)
        nc.sync.dma_start(out=idf_sb, in_=idf)
        nc.sync.dma_start(out=idb_sb, in_=idb)
        nc.vector.memset(eps_sb, LN_EPS)

        # h^T (LayerNormed x, feature-major), Q^T/K^T (feature-major),
        # V (token-major, bf16), O^T (attention output, feature-major)
        ht_sb = big.tile([P, KC, T], F32R, tag="ht")
        qt_sb = big.tile([P, DG // P, T], F32R, tag="qt")
        kt_sb = big.tile([P, DG // P, T], F32R, tag="kt")
        v_sb = big.tile([P, NT, DG], BF16, tag="v")
        ot_sb = big.tile([P, DG // P, T], F32R, tag="ot")

        # ================= Phase A: LayerNorm + transpose =================
        with (
            tc.tile_pool(name="ln", bufs=3) as ln,
            tc.tile_pool(name="lnst", bufs=4) as lnst,
            tc.tile_pool(name="tpp", bufs=4, space="PSUM") as tpp,
        ):
            for tb in range(NT):
                xt = ln.tile([P, D], F32, tag="xt")
                nc.sync.dma_start(out=xt, in_=x[tb * P:(tb + 1) * P, :])

                stats = lnst.tile([P, 2, 6], F32, tag="stats")
                xg = xt.rearrange("p (g d) -> p g d", g=2)
                nc.vector.bn_stats(out=stats[:, 0, :], in_=xg[:, 0, :])
                nc.vector.bn_stats(out=stats[:, 1, :], in_=xg[:, 1, :])
                mv = lnst.tile([P, 2], F32, tag="mv")
                nc.vector.bn_aggr(out=mv, in_=stats)

                rstd = lnst.tile([P, 1], F32, tag="rstd")
                nc.scalar.activation(
                    out=rstd, in_=mv[:, 1:2],
                    func=mybir.ActivationFunctionType.Sqrt,
                    bias=eps_sb, scale=1.0,
                )
                nc.vector.reciprocal(out=rstd, in_=rstd)
                nmr = lnst.tile([P, 1], F32, tag="nmr")
                nc.vector.tensor_mul(nmr, mv[:, 0:1], rstd)
                nc.vector.tensor_scalar_mul(nmr, nmr, -1.0)

                hn = ln.tile([P, D], F32R, tag="hn")
                nc.scalar.activation(
                    out=hn, in_=xt,
                    func=mybir.ActivationFunctionType.Identity,
                    bias=nmr, scale=rstd,
                )
                for c in range(KC):
                    tp = tpp.tile([P, P], F32R, tag="tp")
                    nc.tensor.transpose(
                        _rc(tp), _rc(hn[:, c * P:(c + 1) * P]), _rc(idf_sb))
                    nc.vector.tensor_copy(
                        ht_sb[:, c, tb * P:(tb + 1) * P], tp)

        # ================= Phase B: Q/K/V projections =================
        with tc.tile_pool(name="qkvp", bufs=3, space="PSUM") as qkvp:
            NQ = 512
            for oc in range(DG // P):
                for nt in range(T // NQ):
                    for w_sb, dst, b_sb in ((wq_sb, qt_sb, bq_sb),
                                            (wk_sb, kt_sb, bk_sb)):
                        ps = qkvp.tile([P, NQ], F32, tag="ps")
                        for kc in range(KC):
                            nc.tensor.matmul(
                                ps,
                                _rc(w_sb[:, kc, oc * P:(oc + 1) * P]),
                                _rc(ht_sb[:, kc, nt * NQ:(nt + 1) * NQ]),
                                start=(kc == 0), stop=(kc == KC - 1),
                            )
                        nc.vector.tensor_scalar_add(
                            dst[:, oc, nt * NQ:(nt + 1) * NQ], ps,
                            b_sb[:, oc:oc + 1])
            for tb in range(NT):
                ps = qkvp.tile([P, DG], F32, tag="psv")
                for kc in range(KC):
                    nc.tensor.matmul(
                        ps,
                        _rc(ht_sb[:, kc, tb * P:(tb + 1) * P]),
                        _rc(wv_sb[:, kc, :]),
                        start=(kc == 0), stop=(kc == KC - 1),
                    )
                nc.vector.tensor_add(v_sb[:, tb, :], ps, bv_sb)

        # ================= Phase C: banded attention =================
        with (
            tc.tile_pool(name="sp", bufs=2, space="PSUM") as sp,
            tc.tile_pool(name="ptp", bufs=3, space="PSUM") as ptp,
            tc.tile_pool(name="avp", bufs=2, space="PSUM") as avp,
            tc.tile_pool(name="smx", bufs=3) as smx,
            tc.tile_pool(name="smst", bufs=4) as smst,
        ):
            for oc in range(DG // P):           # output-chunk = head pair
                for qb in range(NT):
                    njb = min(3, qb + 1)
                    jw = njb * P
                    j0 = (qb - njb + 1) * P
                    av = avp.tile([P, P], F32, tag="av")
                    for hh in range(2):         # head within pair
                        p0 = hh * DK
                        qsl = slice(qb * P, (qb + 1) * P)
                        s = sp.tile([P, 3 * P], F32, tag="s")
                        nc.tensor.matmul(
                            s[:, :jw],
                            _rc(qt_sb[p0:p0 + DK, oc, qsl]),
                            _rc(kt_sb[p0:p0 + DK, oc, j0:j0 + jw]),
                            start=True, stop=True,
                        )
                        if njb == 3:
                            nc.vector.tensor_add(
                                s[:, 0:P], s[:, 0:P], mup_sb)
                        nc.vector.tensor_add(
                            s[:, jw - P:jw], s[:, jw - P:jw], mlo_sb)

                        pb = smx.tile([P, 3 * P], BF16, tag="pb")
                        den = smst.tile([P, 1], F32, tag="den")
                        nc.scalar.activation(
                            out=pb[:, :jw], in_=s[:, :jw],
                            func=mybir.ActivationFunctionType.Exp,
                            accum_out=den,
                        )
                        nc.vector.reciprocal(out=den, in_=den)
                        nc.vector.tensor_scalar_mul(
                            pb[:, :jw], pb[:, :jw], den)

                        h = oc * 2 + hh
                        for jj in range(njb):
                            pt = ptp.tile([P, P], BF16, tag="pt")
                            nc.tensor.transpose(
                                pt, pb[:, jj * P:(jj + 1) * P], idb_sb)
                            pts = smx.tile([P, P], BF16, tag="pts")
                            nc.vector.tensor_copy(pts, pt)
                            jb = qb - njb + 1 + jj
                            nc.tensor.matmul(
                                av[p0:p0 + DK, :],
                                v_sb[:, jb, h * DK:(h + 1) * DK],
                                pts,
                                start=(jj == 0), stop=(jj == njb - 1),
                                tile_position=(0, p0),
                            )
                    nc.vector.tensor_copy(ot_sb[:, oc, qsl], av)

        # ================= Phase D: partial out-projection =================
        with (
            tc.tile_pool(name="fpp", bufs=3, space="PSUM") as fpp,
            tc.tile_pool(name="fout", bufs=3) as fout,
        ):
            NO = 512
            for tb in range(NT):
                for on in range(D // NO):
                    ps = fpp.tile([P, NO], F32, tag="ps")
                    for kd in range(DG // P):
                        nc.tensor.matmul(
                            ps,
                            _rc(ot_sb[:, kd, tb * P:(tb + 1) * P]),
                            _rc(wo_sb[:, kd, on * NO:(on + 1) * NO]),
                            start=(kd == 0), stop=(kd == DG // P - 1),
                        )
                    ob = fout.tile([P, NO], F32, tag="ob")
                    nc.vector.tensor_copy(ob, ps)
                    nc.sync.dma_start(
                        out=partial[tb * P:(tb + 1) * P, on * NO:(on + 1) * NO],
                        in_=ob)


def build_nc():
    nc = bacc.Bacc("TRN2", target_bir_lowering=False, debug=False,
                   num_devices=8)
    with tile.TileContext(nc) as tc:
        _body(tc)
    nc.compile()
    return nc


def _prep_core_inputs(x, Wq, Wk, Wv, Wo, gamma, beta):
    """Host-side prep: per-(batch, head-group) input dicts."""
    import ml_dtypes
    B = x.shape[0]
    NEG = np.float32(MASKVAL)
    ii = np.arange(P)[:, None]
    jj = np.arange(P)[None, :]
    mup = np.where(jj > ii, np.float32(0), NEG).astype(np.float32)
    mlo = np.where(jj <= ii, np.float32(0), NEG).astype(np.float32)
    idf = np.eye(P, dtype=np.float32)
    idb = np.eye(P, dtype=np.float32).astype(ml_dtypes.bfloat16)

    in_maps = []
    for b in range(B):
        for g in range(4):
            sl = slice(g * DG, (g + 1) * DG)
            sq = np.float32(1.0 / np.sqrt(DK))
            wq_g = (gamma[:, None] * Wq[:, sl] * sq).astype(np.float32)
            wk_g = (gamma[:, None] * Wk[:, sl]).astype(np.float32)
            wv_g = (gamma[:, None] * Wv[:, sl]).astype(np.float32)
            bq_g = ((beta @ Wq[:, sl]) * sq).astype(np.float32)
            bk_g = (beta @ Wk[:, sl]).astype(np.float32)
            bv_g = (beta @ Wv[:, sl]).astype(np.float32)
            in_maps.append({
                "x": np.ascontiguousarray(x[b]).astype(np.float32),
                "wq": wq_g, "wk": wk_g, "wv": wv_g,
                "wo": np.ascontiguousarray(Wo[sl, :]).astype(np.float32),
                "bq": np.ascontiguousarray(bq_g.reshape(DG // P, P).T),
                "bk": np.ascontiguousarray(bk_g.reshape(DG // P, P).T),
                "bv": np.tile(bv_g[None, :], (P, 1)),
                "mup": mup, "mlo": mlo, "idf": idf, "idb": idb,
            })
    return in_maps


def _ntff_hook(so_path="/opt/axon/libaxon_pjrt.so"):
    import contextlib
    import ctypes

    lib = ctypes.CDLL(so_path)
    lib.axon_start_nrt_profile.argtypes = [
        ctypes.POINTER(ctypes.c_int64), ctypes.c_size_t]
    lib.axon_start_nrt_profile.restype = ctypes.c_int64
    lib.axon_stop_nrt_profile.argtypes = [ctypes.c_char_p]
    lib.axon_stop_nrt_profile.restype = ctypes.c_int64

    @contextlib.contextmanager
    def _hook(output_dir, device_ids):
        import jax
        jax.devices()
        if device_ids:
            ids = (ctypes.c_int64 * len(device_ids))(*device_ids)
            rc = lib.axon_start_nrt_profile(ids, len(device_ids))
        else:
            rc = lib.axon_start_nrt_profile(None, 0)
        if rc != 0:
            raise RuntimeError(f"axon_start_nrt_profile rc={rc}")
        try:
            yield
        finally:
            n = lib.axon_stop_nrt_profile(str(output_dir).encode())
            print(f"profile: {n} file(s) written to {output_dir}")

    return _hook


def _run_traced(nc, in_maps, trace_dir=None):
    """Execute via PJRT with NTFF capture; return BassKernelResults with
    exec_time_ns and a perfetto trace."""
    import glob
    import tempfile

    import gauge.profiler
    from concourse import bass2jax, bass_utils
    from concourse._compat import FishPath

    neff_dir = trace_dir or tempfile.mkdtemp(prefix="trn_trace_")
    hook = _ntff_hook()
    with hook(neff_dir, [0]):
        results = bass2jax.run_bass_via_pjrt(nc, in_maps, n_cores=len(in_maps))

    ntffs = glob.glob(os.path.join(neff_dir, "*_body*.ntff"))
    if not ntffs:
        print(f"no ntffs in {neff_dir}: {os.listdir(neff_dir)}")
        return bass_utils.BassKernelResults(
            results=results, instructions_and_trace=None,
            profile_json=None, exec_time_ns=None)

    profile = gauge.profiler.Profile(
        profile_path=FishPath(neff_dir),
        kernel_dev_mode=True,
        profile_on_exit=False,
        bass_kernel=nc.m,
        offline_processing=True,
        fname="*_body*",
        metadata={},
    )
    return bass_utils._process_ntff_profile(
        profile, neff_dir, nc, list(range(len(in_maps))),
        None, False, {}, trace_events=False,
    ).as_bass_kernel_results(results)


def kernel(x, Wq, Wk, Wv, Wo, bo, gamma, beta, trace=False):
    global LAST_PROFILE
    x = np.asarray(x, dtype=np.float32)
    Wq, Wk, Wv, Wo = (np.asarray(a, dtype=np.float32) for a in (Wq, Wk, Wv, Wo))
    bo = np.asarray(bo, dtype=np.float32)
    gamma = np.asarray(gamma, dtype=np.float32)
    beta = np.asarray(beta, dtype=np.float32)

    nc = build_nc()
    in_maps = _prep_core_inputs(x, Wq, Wk, Wv, Wo, gamma, beta)
    if trace:
        res = _run_traced(nc, in_maps)
    else:
        res = run_bass_kernel_spmd(nc, in_maps, core_ids=list(range(8)))
    LAST_PROFILE = {"exec_time_ns": res.exec_time_ns}

    B = x.shape[0]
    out = np.empty_like(x)
    for b in range(B):
        acc = x[b] + bo[None, :]
        for g in range(4):
            acc = acc + res.results[b * 4 + g]["partial"]
        out[b] = acc
    return out



# revision 22
# speedup vs baseline: 1.1179x; 1.1179x over previous
"""Local causal (sliding-window) attention block on 8 TRN2 NeuronCores.

Reference computation (per batch b):
    h = LayerNorm(x) * gamma + beta
    Q = h@Wq, K = h@Wk, V = h@Wv          (heads: 16 x 64)
    S = QK^T/sqrt(dk) masked to causal band of width 256
    out = x + softmax(S)@V @ Wo + bo

Sharding: 8 cores = 2 batches x 4 head-groups (4 heads each).
Each core computes LN(x_b), its head-group's Q/K/V, banded attention,
and a partial out-projection  attn_g @ Wo[g]  (token-major, [T, D]).
Host reduces: out[b] = x[b] + sum_g partial[b,g] + bo.

Implementation notes (v2, overhead-optimized):
- All matmuls run in bf16 (fp32 PSUM accumulation).
- h^T is produced by the DMA XBAR transpose (dma_start(transpose=True)),
  eliminating all PE transposes.
- Attention computes S^T[k, q] tiles directly (k on partitions), so the
  probability tiles feed P@V without any transpose; the softmax
  denominator comes from a ones-column appended to V, and the final
  1/den scaling uses a stride-0 DMA broadcast + one Pool multiply.
- Elementwise work is spread across DVE / Act / Pool to keep the PE the
  only near-saturated engine.
"""

import os

import numpy as np

import concourse.bass as bass
import concourse.tile as tile
from concourse import bacc, mybir
from concourse.bass_utils import run_bass_kernel_spmd

F32 = mybir.dt.float32
F32R = mybir.dt.float32r
BF16 = mybir.dt.bfloat16

T = 2048          # tokens per batch
D = 1024          # model dim
HG = 4            # heads per core
DK = 64           # head dim
DG = HG * DK      # head-group feature width (256)
WIN = 256         # attention window
P = 128           # partitions
NT = T // P       # 16 token tiles
KC = D // P       # 8 feature chunks
NG = NT // 4      # 4 query groups of 512 tokens
LN_EPS = 1e-5
MASKVAL = -1e9

# filled by test.py via run(trace=True)
LAST_PROFILE = {}


def _nq(kb):
    return min(3 * P, (NT - kb) * P)


def _body(tc):
    nc = tc.nc

    x = nc.dram_tensor("x", [T, D], BF16, kind="ExternalInput").ap()
    wq = nc.dram_tensor("wq", [P, KC, DG], BF16, kind="ExternalInput").ap()
    wk = nc.dram_tensor("wk", [P, KC, DG], BF16, kind="ExternalInput").ap()
    wv = nc.dram_tensor("wv", [P, KC, DG], BF16, kind="ExternalInput").ap()
    wo = nc.dram_tensor("wo", [P, DG // P, D], BF16, kind="ExternalInput").ap()
    bq = nc.dram_tensor("bq", [P, DG // P], F32, kind="ExternalInput").ap()
    bk = nc.dram_tensor("bk", [P, DG // P], F32, kind="ExternalInput").ap()
    bv = nc.dram_tensor("bv", [P, DG], F32, kind="ExternalInput").ap()
    md2 = nc.dram_tensor("md2", [P, 2, P], BF16, kind="ExternalInput").ap()
    mf2 = nc.dram_tensor("mf2", [P, 2, P], BF16, kind="ExternalInput").ap()
    vones = nc.dram_tensor("vones", [P, NT, HG], BF16, kind="ExternalInput").ap()
    partial = nc.dram_tensor("partial", [T, D], BF16, kind="ExternalOutput").ap()
    dbg = os.environ.get("KDEBUG", "") == "1"
    dscr = nc.dram_tensor("dscr", [NG * HG, 512], F32,
                          kind="ExternalOutput" if dbg else "Internal").ap()
    if dbg:
        d_ht = nc.dram_tensor("d_ht", [P, KC, T], BF16, kind="ExternalOutput").ap()
        d_qt = nc.dram_tensor("d_qt", [P, 2, T], BF16, kind="ExternalOutput").ap()
        d_kt = nc.dram_tensor("d_kt", [P, 2, T], BF16, kind="ExternalOutput").ap()
        d_v = nc.dram_tensor("d_v", [P, NT, HG * (DK + 1)], BF16, kind="ExternalOutput").ap()
        d_ot = nc.dram_tensor("d_ot", [P, 2, T], BF16, kind="ExternalOutput").ap()

    with (
        tc.tile_pool(name="consts", bufs=1) as consts,
        tc.tile_pool(name="big", bufs=1) as big,
    ):
        # ---- resident SBUF tensors ----
        wq_sb = consts.tile([P, KC, DG], BF16, tag="wq")
        wk_sb = consts.tile([P, KC, DG], BF16, tag="wk")
        wv_sb = consts.tile([P, KC, DG], BF16, tag="wv")
        wo_sb = consts.tile([P, DG // P, D], BF16, tag="wo")
        bq_sb = consts.tile([P, DG // P], F32, tag="bq")
        bk_sb = consts.tile([P, DG // P], F32, tag="bk")
        bv_sb = consts.tile([P, DG], F32, tag="bv")
        md2_sb = consts.tile([P, 2, P], BF16, tag="md2")
        mf2_sb = consts.tile([P, 2, P], BF16, tag="mf2")
        eps_sb = consts.tile([P, 1], F32, tag="eps")

        nc.sync.dma_start(out=wq_sb, in_=wq)
        nc.sync.dma_start(out=wk_sb, in_=wk)
        nc.sync.dma_start(out=wv_sb, in_=wv)
        nc.sync.dma_start(out=wo_sb, in_=wo)
        nc.sync.dma_start(out=bq_sb, in_=bq)
        nc.sync.dma_start(out=bk_sb, in_=bk)
        nc.sync.dma_start(out=bv_sb, in_=bv)
        nc.sync.dma_start(out=md2_sb, in_=md2)
        nc.sync.dma_start(out=mf2_sb, in_=mf2)
        nc.vector.memset(eps_sb, LN_EPS)

        # h^T (feature-major), Q^T/K^T (feature-major), V (token-major,
        # with a ones column per head for the softmax denominator),
        # O^T (attention output, feature-major)
        ht_sb = big.tile([P, KC, T], BF16, tag="ht")
        qt_sb = big.tile([P, DG // P, T], BF16, tag="qt")
        kt_sb = big.tile([P, DG // P, T], BF16, tag="kt")
        v_sb = big.tile([P, NT, HG * (DK + 1)], BF16, tag="v")
        ot_sb = big.tile([P, DG // P, T], BF16, tag="ot")

        # ones columns of V (denominator trick)
        nc.sync.dma_start(out=v_sb[:, :, DK::DK + 1], in_=vones)

        # ============ Front: LayerNorm + h^T + Q/K/V projections ============
        with (
            tc.tile_pool(name="xp", bufs=3) as xp,
            tc.tile_pool(name="hp", bufs=3) as hp,
            tc.tile_pool(name="lnst", bufs=4) as lnst,
            tc.tile_pool(name="qkp", bufs=2, space="PSUM") as qkp,
            tc.tile_pool(name="vp", bufs=2, space="PSUM") as vp,
        ):
            for tb in range(NT):
                ts = slice(tb * P, (tb + 1) * P)
                xt = xp.tile([P, D], BF16, tag="xt")
                nc.sync.dma_start(out=xt, in_=x[ts, :])

                stats = lnst.tile([P, 2, 6], F32, tag="stats")
                xg = xt.rearrange("p (g d) -> p g d", g=2)
                nc.vector.bn_stats(out=stats[:, 0, :], in_=xg[:, 0, :])
                nc.vector.bn_stats(out=stats[:, 1, :], in_=xg[:, 1, :])
                mv = lnst.tile([P, 2], F32, tag="mv")
                nc.vector.bn_aggr(out=mv, in_=stats)

                rstd = lnst.tile([P, 1], F32, tag="rstd")
                nc.scalar.activation(
                    out=rstd, in_=mv[:, 1:2],
                    func=mybir.ActivationFunctionType.Sqrt,
                    bias=eps_sb, scale=1.0,
                )
                nc.vector.reciprocal(out=rstd, in_=rstd)
                nmr = lnst.tile([P, 1], F32, tag="nmr")
                nc.vector.tensor_scalar(
                    out=nmr, in0=mv[:, 0:1], scalar1=rstd, scalar2=-1.0,
                    op0=mybir.AluOpType.mult, op1=mybir.AluOpType.mult,
                )

                hn = hp.tile([P, D], BF16, tag="hn")
                nc.scalar.activation(
                    out=hn, in_=xt,
                    func=mybir.ActivationFunctionType.Identity,
                    bias=nmr, scale=rstd,
                )
                # h^T via DMA XBAR transpose: ht[p, c, t] = hn[t, c*128+p]
                nc.sync.dma_start(out=ht_sb[:, :, ts], in_=hn, transpose=True)

                # V projection for this token tile (token-major)
                ps = vp.tile([P, DG], F32, tag="psv")
                for kc in range(KC):
                    nc.tensor.matmul(
                        ps, ht_sb[:, kc, ts], wv_sb[:, kc, :],
                        start=(kc == 0), stop=(kc == KC - 1),
                    )
                nc.vector.tensor_add(
                    v_sb[:, tb, :].rearrange("p (h d) -> p h d", d=DK + 1)[:, :, 0:DK],
                    ps.rearrange("p (h d) -> p h d", d=DK),
                    bv_sb.rearrange("p (h d) -> p h d", d=DK),
                )

                # Q^T / K^T per completed 512-token slice
                if tb % 4 == 3:
                    sl = tb // 4
                    ss = slice(sl * 512, (sl + 1) * 512)
                    for w_sb, b_sb, dst in ((wq_sb, bq_sb, qt_sb),
                                            (wk_sb, bk_sb, kt_sb)):
                        for oc in range(DG // P):
                            pq = qkp.tile([P, 512], F32, tag="psqk")
                            for kc in range(KC):
                                nc.tensor.matmul(
                                    pq,
                                    w_sb[:, kc, oc * P:(oc + 1) * P],
                                    ht_sb[:, kc, ss],
                                    start=(kc == 0), stop=(kc == KC - 1),
                                )
                            nc.scalar.activation(
                                out=dst[:, oc, ss], in_=pq,
                                func=mybir.ActivationFunctionType.Identity,
                                bias=b_sb[:, oc:oc + 1], scale=1.0)

        # ============ Attention (S^T formulation) + out-projection ============
        with (
            tc.tile_pool(name="sp", bufs=2, space="PSUM") as sp,
            tc.tile_pool(name="avp", bufs=2, space="PSUM") as avp,
            tc.tile_pool(name="fpp", bufs=2, space="PSUM") as fpp,
            tc.tile_pool(name="ep", bufs=16) as ep,
            tc.tile_pool(name="rp", bufs=3) as rp,
            tc.tile_pool(name="bp", bufs=3) as bp,
            tc.tile_pool(name="op", bufs=3) as op,
        ):
            et_ref = {}

            def st_pair(g, kb, oc):
                """One S^T + mask + exp tile for 2 heads of chunk oc."""
                nq = _nq(kb)
                ks = slice(kb * P, (kb + 1) * P)
                s2 = sp.tile([P, 2, 512], F32, tag="s2")
                for hh in range(2):
                    p0 = hh * DK
                    nc.tensor.matmul(
                        s2[:, hh, 0:nq],
                        kt_sb[p0:p0 + DK, oc, ks],
                        qt_sb[p0:p0 + DK, oc, kb * P:kb * P + nq],
                        start=True, stop=True,
                    )
                et = ep.tile([P, 2, 3 * P], BF16, tag="et")
                nc.scalar.activation(
                    out=et[:, :, 0:nq], in_=s2[:, :, 0:nq],
                    func=mybir.ActivationFunctionType.Exp,
                )
                # band mask: zero out-of-band probabilities (0/1 bf16)
                nc.gpsimd.tensor_mul(et[:, :, 0:P], et[:, :, 0:P], md2_sb)
                if nq == 3 * P:
                    nc.gpsimd.tensor_mul(
                        et[:, :, 2 * P:3 * P], et[:, :, 2 * P:3 * P], mf2_sb)
                et_ref[(oc, kb)] = et

            def pv_head(g, h):
                """Accumulate P@V for one head over query group g."""
                q0 = g * 512
                oc, hh = h // 2, h % 2
                av = avp.tile([DK + 1, 512], F32, tag="av")
                nc.vector.memset(av, 0.0)
                segs = []
                for kb in range(max(0, 4 * g - 2), 4 * g + 4):
                    a = max(kb * P, q0)
                    b2 = min(kb * P + _nq(kb), q0 + 512)
                    segs.append((kb, a - q0, b2 - q0, False))
                for i, (kb, a, b2, init) in enumerate(segs):
                    nc.tensor.matmul(
                        av[:, a:b2],
                        v_sb[:, kb, h * (DK + 1):(h + 1) * (DK + 1)],
                        et_ref[(oc, kb)][:, hh, q0 + a - kb * P:q0 + b2 - kb * P],
                        start=init, stop=(i == len(segs) - 1),
                        skip_group_check=True,
                    )
                rden = rp.tile([1, 512], F32, tag="rden")
                nc.vector.reciprocal(out=rden, in_=av[DK:DK + 1, :])
                i = g * HG + h
                w = nc.sync.dma_start(out=dscr[i:i + 1, :], in_=rden)
                tc.chain_iter_dep(f"dscr{i}", w.ins)
                bc = bp.tile([DK, 512], F32, tag="bc")
                r = nc.sync.dma_start(
                    out=bc, in_=dscr[i:i + 1, :].to_broadcast([DK, 512]))
                tc.chain_iter_dep(f"dscr{i}", r.ins)
                return av, bc

            def norm_head(g, h, av, bc):
                """Scale by the broadcast 1/den into O^T."""
                q0 = g * 512
                oc, hh = h // 2, h % 2
                nc.vector.tensor_mul(
                    ot_sb[hh * DK:(hh + 1) * DK, oc, q0:q0 + 512],
                    av[0:DK, :], bc)

            def outproj(tb):
                ts = slice(tb * P, (tb + 1) * P)
                ob = op.tile([P, D], BF16, tag="ob")
                for on in range(2):
                    po = fpp.tile([P, 512], F32, tag="po")
                    for kd in range(DG // P):
                        nc.tensor.matmul(
                            po,
                            ot_sb[:, kd, ts],
                            wo_sb[:, kd, on * 512:(on + 1) * 512],
                            start=(kd == 0), stop=(kd == DG // P - 1),
                        )
                    if (tb + on) % 2 == 0:
                        nc.scalar.activation(
                            out=ob[:, on * 512:(on + 1) * 512], in_=po,
                            func=mybir.ActivationFunctionType.Identity,
                            scale=1.0)
                    else:
                        nc.vector.tensor_copy(ob[:, on * 512:(on + 1) * 512], po)
                nc.sync.dma_start(out=partial[ts, :], in_=ob)

            for kb in range(0, 4):
                for oc in range(2):
                    st_pair(0, kb, oc)
            for g in range(NG):
                # P@V for this group; 1/den broadcasts lag one head so the
                # PE never waits on the DVE reciprocal
                pend = None
                for h in range(HG):
                    cur = (g, h, *pv_head(g, h))
                    if pend is not None:
                        norm_head(*pend)
                    pend = cur
                norm_head(*pend)
                # interleave next group's scores with this group's
                # out-projection to keep PE fed while Act runs the exps
                seq = []
                if g + 1 < NG:
                    seq = [("st", 4 * (g + 1) + j, oc)
                           for j in range(4) for oc in range(2)]
                ops = [("op", 4 * g + j, None) for j in range(4)]
                merged = []
                si, oi = 0, 0
                order = ["st", "st", "op", "st", "op", "st", "op", "st",
                         "op", "st", "st", "st"]
                for kind in order:
                    if kind == "st" and si < len(seq):
                        merged.append(seq[si]); si += 1
                    elif kind == "op" and oi < len(ops):
                        merged.append(ops[oi]); oi += 1
                merged += seq[si:] + ops[oi:]
                for kind, a, b in merged:
                    if kind == "st":
                        st_pair(g + 1, a, b)
                    else:
                        outproj(a)

            if dbg:
                nc.sync.dma_start(out=d_ht, in_=ht_sb)
                nc.sync.dma_start(out=d_qt, in_=qt_sb)
                nc.sync.dma_start(out=d_kt, in_=kt_sb)
                nc.sync.dma_start(out=d_v, in_=v_sb)
                nc.sync.dma_start(out=d_ot, in_=ot_sb)


def build_nc():
    nc = bacc.Bacc("TRN2", target_bir_lowering=False, debug=False,
                   num_devices=8)
    with tile.TileContext(nc) as tc:
        _body(tc)
    nc.compile()
    return nc


def _prep_core_inputs(x, Wq, Wk, Wv, Wo, gamma, beta):
    """Host-side prep: per-(batch, head-group) input dicts."""
    import ml_dtypes
    BF = ml_dtypes.bfloat16
    B = x.shape[0]
    kk = np.arange(P)[:, None]
    qq = np.arange(P)[None, :]
    md = (kk <= qq).astype(BF)
    mf = (kk > qq).astype(BF)
    md2 = np.ascontiguousarray(np.stack([md, md], axis=1))
    mf2 = np.ascontiguousarray(np.stack([mf, mf], axis=1))

    def fold(w):
        # [D, DG] -> [128, KC, DG] with d = c*128 + p
        return np.ascontiguousarray(
            w.reshape(KC, P, DG).transpose(1, 0, 2)).astype(BF)

    in_maps = []
    for b in range(B):
        for g in range(4):
            sl = slice(g * DG, (g + 1) * DG)
            sq = np.float32(1.0 / np.sqrt(DK))
            wq_g = fold(gamma[:, None] * Wq[:, sl] * sq)
            wk_g = fold(gamma[:, None] * Wk[:, sl])
            wv_g = fold(gamma[:, None] * Wv[:, sl])
            wo_g = np.ascontiguousarray(
                Wo[sl, :].reshape(DG // P, P, D).transpose(1, 0, 2)).astype(BF)
            bq_g = ((beta @ Wq[:, sl]) * sq).astype(np.float32)
            bk_g = (beta @ Wk[:, sl]).astype(np.float32)
            bv_g = (beta @ Wv[:, sl]).astype(np.float32)
            in_maps.append({
                "x": np.ascontiguousarray(x[b]).astype(BF),
                "wq": wq_g, "wk": wk_g, "wv": wv_g, "wo": wo_g,
                "bq": np.ascontiguousarray(bq_g.reshape(DG // P, P).T),
                "bk": np.ascontiguousarray(bk_g.reshape(DG // P, P).T),
                "bv": np.tile(bv_g[None, :], (P, 1)),
                "md2": md2, "mf2": mf2,
                "vones": np.ones((P, NT, HG), dtype=BF),
            })
    return in_maps


def _ntff_hook(so_path="/opt/axon/libaxon_pjrt.so"):
    import contextlib
    import ctypes

    lib = ctypes.CDLL(so_path)
    lib.axon_start_nrt_profile.argtypes = [
        ctypes.POINTER(ctypes.c_int64), ctypes.c_size_t]
    lib.axon_start_nrt_profile.restype = ctypes.c_int64
    lib.axon_stop_nrt_profile.argtypes = [ctypes.c_char_p]
    lib.axon_stop_nrt_profile.restype = ctypes.c_int64

    @contextlib.contextmanager
    def _hook(output_dir, device_ids):
        import jax
        jax.devices()
        if device_ids:
            ids = (ctypes.c_int64 * len(device_ids))(*device_ids)
            rc = lib.axon_start_nrt_profile(ids, len(device_ids))
        else:
            rc = lib.axon_start_nrt_profile(None, 0)
        if rc != 0:
            raise RuntimeError(f"axon_start_nrt_profile rc={rc}")
        try:
            yield
        finally:
            n = lib.axon_stop_nrt_profile(str(output_dir).encode())
            print(f"profile: {n} file(s) written to {output_dir}")

    return _hook


def _run_traced(nc, in_maps, trace_dir=None):
    """Execute via PJRT with NTFF capture; return BassKernelResults with
    exec_time_ns and a perfetto trace."""
    import glob
    import tempfile

    import gauge.profiler
    from concourse import bass2jax, bass_utils
    from concourse._compat import FishPath

    neff_dir = trace_dir or tempfile.mkdtemp(prefix="trn_trace_")
    hook = _ntff_hook()
    with hook(neff_dir, [0]):
        results = bass2jax.run_bass_via_pjrt(nc, in_maps, n_cores=len(in_maps))

    ntffs = glob.glob(os.path.join(neff_dir, "*_body*.ntff"))
    if not ntffs:
        print(f"no ntffs in {neff_dir}: {os.listdir(neff_dir)}")
        return bass_utils.BassKernelResults(
            results=results, instructions_and_trace=None,
            profile_json=None, exec_time_ns=None)

    profile = gauge.profiler.Profile(
        profile_path=FishPath(neff_dir),
        kernel_dev_mode=True,
        profile_on_exit=False,
        bass_kernel=nc.m,
        offline_processing=True,
        fname="*_body*",
        metadata={},
    )
    return bass_utils._process_ntff_profile(
        profile, neff_dir, nc, list(range(len(in_maps))),
        None, False, {}, trace_events=False,
    ).as_bass_kernel_results(results)


def kernel(x, Wq, Wk, Wv, Wo, bo, gamma, beta, trace=False):
    global LAST_PROFILE
    x = np.asarray(x, dtype=np.float32)
    Wq, Wk, Wv, Wo = (np.asarray(a, dtype=np.float32) for a in (Wq, Wk, Wv, Wo))
    bo = np.asarray(bo, dtype=np.float32)
    gamma = np.asarray(gamma, dtype=np.float32)
    beta = np.asarray(beta, dtype=np.float32)

    nc = build_nc()
    in_maps = _prep_core_inputs(x, Wq, Wk, Wv, Wo, gamma, beta)
    if trace:
        res = _run_traced(nc, in_maps)
    else:
        res = run_bass_kernel_spmd(nc, in_maps, core_ids=list(range(8)))
    LAST_PROFILE = {"exec_time_ns": res.exec_time_ns}

    B = x.shape[0]
    out = np.empty_like(x)
    for b in range(B):
        acc = x[b] + bo[None, :]
        for g in range(4):
            acc = acc + np.asarray(res.results[b * 4 + g]["partial"],
                                   dtype=np.float32)
        out[b] = acc
    return out


# revision 23
# speedup vs baseline: 1.1644x; 1.0416x over previous
"""Local causal (sliding-window) attention block on 8 TRN2 NeuronCores.

Reference computation (per batch b):
    h = LayerNorm(x) * gamma + beta
    Q = h@Wq, K = h@Wk, V = h@Wv          (heads: 16 x 64)
    S = QK^T/sqrt(dk) masked to causal band of width 256
    out = x + softmax(S)@V @ Wo + bo

Sharding: 8 cores = 2 batches x 4 head-groups (4 heads each).
Each core computes LN(x_b), its head-group's Q/K/V, banded attention,
and a partial out-projection  attn_g @ Wo[g]  (token-major, [T, D]).
Host reduces: out[b] = x[b] + sum_g partial[b,g] + bo.

Implementation notes (v2, overhead-optimized):
- All matmuls run in bf16 (fp32 PSUM accumulation).
- h^T is produced by the DMA XBAR transpose (dma_start(transpose=True)),
  eliminating all PE transposes.
- Attention computes S^T[k, q] tiles directly (k on partitions), so the
  probability tiles feed P@V without any transpose; the softmax
  denominator comes from a ones-column appended to V, and the final
  1/den scaling uses a stride-0 DMA broadcast + one Pool multiply.
- Elementwise work is spread across DVE / Act / Pool to keep the PE the
  only near-saturated engine.
"""

import os

import numpy as np

import concourse.bass as bass
import concourse.tile as tile
from concourse import bacc, mybir
from concourse.bass_utils import run_bass_kernel_spmd

F32 = mybir.dt.float32
F32R = mybir.dt.float32r
BF16 = mybir.dt.bfloat16

T = 2048          # tokens per batch
D = 1024          # model dim
HG = 4            # heads per core
DK = 64           # head dim
DG = HG * DK      # head-group feature width (256)
WIN = 256         # attention window
P = 128           # partitions
NT = T // P       # 16 token tiles
KC = D // P       # 8 feature chunks
NG = NT // 4      # 4 query groups of 512 tokens
LN_EPS = 1e-5
MASKVAL = -1e9

# filled by test.py via run(trace=True)
LAST_PROFILE = {}


def _nq(kb):
    return min(3 * P, (NT - kb) * P)


def _body(tc):
    nc = tc.nc

    x = nc.dram_tensor("x", [T, D], BF16, kind="ExternalInput").ap()
    wq = nc.dram_tensor("wq", [P, KC, DG], BF16, kind="ExternalInput").ap()
    wk = nc.dram_tensor("wk", [P, KC, DG], BF16, kind="ExternalInput").ap()
    wv = nc.dram_tensor("wv", [P, KC, DG], BF16, kind="ExternalInput").ap()
    wo = nc.dram_tensor("wo", [P, DG // P, D], BF16, kind="ExternalInput").ap()
    bq = nc.dram_tensor("bq", [P, DG // P], F32, kind="ExternalInput").ap()
    bk = nc.dram_tensor("bk", [P, DG // P], F32, kind="ExternalInput").ap()
    bv = nc.dram_tensor("bv", [P, DG], F32, kind="ExternalInput").ap()
    mc = nc.dram_tensor("mc", [P, 2, 3 * P], BF16, kind="ExternalInput").ap()
    zz = nc.dram_tensor("zz", [1, DK + 1 + 512], BF16, kind="ExternalInput").ap()
    vones = nc.dram_tensor("vones", [P, NT, HG], BF16, kind="ExternalInput").ap()
    partial = nc.dram_tensor("partial", [T, D], BF16, kind="ExternalOutput").ap()
    dbg = os.environ.get("KDEBUG", "") == "1"
    dscr = nc.dram_tensor("dscr", [NG * HG, 512], F32,
                          kind="ExternalOutput" if dbg else "Internal").ap()
    dscr2 = nc.dram_tensor("dscr2", [NG * HG, 512], F32, kind="Internal").ap()
    if dbg:
        d_ht = nc.dram_tensor("d_ht", [P, KC, T], BF16, kind="ExternalOutput").ap()
        d_qt = nc.dram_tensor("d_qt", [P, 2, T], BF16, kind="ExternalOutput").ap()
        d_kt = nc.dram_tensor("d_kt", [P, 2, T], BF16, kind="ExternalOutput").ap()
        d_v = nc.dram_tensor("d_v", [P, NT, HG * (DK + 1)], BF16, kind="ExternalOutput").ap()
        d_ot = nc.dram_tensor("d_ot", [P, 2, T], BF16, kind="ExternalOutput").ap()

    with (
        tc.tile_pool(name="consts", bufs=1) as consts,
        tc.tile_pool(name="big", bufs=1) as big,
    ):
        # ---- resident SBUF tensors ----
        wq_sb = consts.tile([P, KC, DG], BF16, tag="wq")
        wk_sb = consts.tile([P, KC, DG], BF16, tag="wk")
        wv_sb = consts.tile([P, KC, DG], BF16, tag="wv")
        wo_sb = consts.tile([P, DG // P, D], BF16, tag="wo")
        bq_sb = consts.tile([P, DG // P], F32, tag="bq")
        bk_sb = consts.tile([P, DG // P], F32, tag="bk")
        bv_sb = consts.tile([P, DG], F32, tag="bv")
        mc_sb = consts.tile([P, 2, 3 * P], BF16, tag="mc")
        zz_sb = consts.tile([1, DK + 1 + 512], BF16, tag="zz")
        eps_sb = consts.tile([P, 1], F32, tag="eps")

        nc.sync.dma_start(out=wq_sb, in_=wq)
        nc.sync.dma_start(out=wk_sb, in_=wk)
        nc.sync.dma_start(out=wv_sb, in_=wv)
        nc.sync.dma_start(out=wo_sb, in_=wo)
        nc.sync.dma_start(out=bq_sb, in_=bq)
        nc.sync.dma_start(out=bk_sb, in_=bk)
        nc.sync.dma_start(out=bv_sb, in_=bv)
        nc.sync.dma_start(out=mc_sb, in_=mc)
        nc.sync.dma_start(out=zz_sb, in_=zz)
        nc.vector.memset(eps_sb, LN_EPS)

        # h^T (feature-major), Q^T/K^T (feature-major), V (token-major,
        # with a ones column per head for the softmax denominator),
        # O^T (attention output, feature-major)
        ht_sb = big.tile([P, KC, T], BF16, tag="ht")
        qt_sb = big.tile([P, DG // P, T], BF16, tag="qt")
        kt_sb = big.tile([P, DG // P, T], BF16, tag="kt")
        v_sb = big.tile([P, NT, HG * (DK + 1)], BF16, tag="v")
        ot_sb = big.tile([P, DG // P, T], BF16, tag="ot")

        # ones columns of V (denominator trick)
        nc.sync.dma_start(out=v_sb[:, :, DK::DK + 1], in_=vones)

        # ============ Front: LayerNorm + h^T + Q/K/V projections ============
        with (
            tc.tile_pool(name="xp", bufs=3) as xp,
            tc.tile_pool(name="hp", bufs=3) as hp,
            tc.tile_pool(name="lnst", bufs=4) as lnst,
            tc.tile_pool(name="qkp", bufs=2, space="PSUM") as qkp,
            tc.tile_pool(name="vp", bufs=2, space="PSUM") as vp,
        ):
            for tb in range(NT):
                ts = slice(tb * P, (tb + 1) * P)
                xt = xp.tile([P, D], BF16, tag="xt")
                nc.sync.dma_start(out=xt, in_=x[ts, :])

                stats = lnst.tile([P, 2, 6], F32, tag="stats")
                xg = xt.rearrange("p (g d) -> p g d", g=2)
                nc.vector.bn_stats(out=stats[:, 0, :], in_=xg[:, 0, :])
                nc.vector.bn_stats(out=stats[:, 1, :], in_=xg[:, 1, :])
                mv = lnst.tile([P, 2], F32, tag="mv")
                nc.vector.bn_aggr(out=mv, in_=stats)

                rstd = lnst.tile([P, 1], F32, tag="rstd")
                nc.scalar.activation(
                    out=rstd, in_=mv[:, 1:2],
                    func=mybir.ActivationFunctionType.Sqrt,
                    bias=eps_sb, scale=1.0,
                )
                nc.vector.reciprocal(out=rstd, in_=rstd)
                nmr = lnst.tile([P, 1], F32, tag="nmr")
                nc.vector.tensor_scalar(
                    out=nmr, in0=mv[:, 0:1], scalar1=rstd, scalar2=-1.0,
                    op0=mybir.AluOpType.mult, op1=mybir.AluOpType.mult,
                )

                hn = hp.tile([P, D], BF16, tag="hn")
                nc.gpsimd.tensor_scalar(
                    out=hn, in0=xt, scalar1=rstd, scalar2=nmr,
                    op0=mybir.AluOpType.mult, op1=mybir.AluOpType.add,
                )
                # h^T via DMA XBAR transpose: ht[p, c, t] = hn[t, c*128+p]
                nc.sync.dma_start(out=ht_sb[:, :, ts], in_=hn, transpose=True)

                # V projection for this token tile (token-major)
                ps = vp.tile([P, DG], F32, tag="psv")
                for kc in range(KC):
                    nc.tensor.matmul(
                        ps, ht_sb[:, kc, ts], wv_sb[:, kc, :],
                        start=(kc == 0), stop=(kc == KC - 1),
                    )
                nc.vector.tensor_add(
                    v_sb[:, tb, :].rearrange("p (h d) -> p h d", d=DK + 1)[:, :, 0:DK],
                    ps.rearrange("p (h d) -> p h d", d=DK),
                    bv_sb.rearrange("p (h d) -> p h d", d=DK),
                )

                # Q^T / K^T per completed 512-token slice
                if tb % 4 == 3:
                    sl = tb // 4
                    ss = slice(sl * 512, (sl + 1) * 512)
                    for w_sb, b_sb, dst in ((wq_sb, bq_sb, qt_sb),
                                            (wk_sb, bk_sb, kt_sb)):
                        for oc in range(DG // P):
                            pq = qkp.tile([P, 512], F32, tag="psqk")
                            for kc in range(KC):
                                nc.tensor.matmul(
                                    pq,
                                    w_sb[:, kc, oc * P:(oc + 1) * P],
                                    ht_sb[:, kc, ss],
                                    start=(kc == 0), stop=(kc == KC - 1),
                                )
                            nc.vector.tensor_scalar_add(
                                dst[:, oc, ss], pq, b_sb[:, oc:oc + 1])

        # ============ Attention (S^T formulation) + out-projection ============
        with (
            tc.tile_pool(name="sp", bufs=2, space="PSUM") as sp,
            tc.tile_pool(name="avp", bufs=2, space="PSUM") as avp,
            tc.tile_pool(name="fpp", bufs=2, space="PSUM") as fpp,
            tc.tile_pool(name="ep", bufs=16) as ep,
            tc.tile_pool(name="rp", bufs=3) as rp,
            tc.tile_pool(name="bp", bufs=3) as bp,
            tc.tile_pool(name="op", bufs=3) as op,
        ):
            et_ref = {}

            def st_pair(g, kb, oc):
                """One S^T + mask + exp tile for 2 heads of chunk oc."""
                nq = _nq(kb)
                ks = slice(kb * P, (kb + 1) * P)
                s2 = sp.tile([P, 2, 512], F32, tag="s2")
                for hh in range(2):
                    p0 = hh * DK
                    nc.tensor.matmul(
                        s2[:, hh, 0:nq],
                        kt_sb[p0:p0 + DK, oc, ks],
                        qt_sb[p0:p0 + DK, oc, kb * P:kb * P + nq],
                        start=True, stop=True,
                    )
                et = ep.tile([P, 2, 3 * P], BF16, tag="et")
                nc.scalar.activation(
                    out=et[:, :, 0:nq], in_=s2[:, :, 0:nq],
                    func=mybir.ActivationFunctionType.Exp,
                )
                # band mask: zero out-of-band probabilities (0/1 bf16)
                nc.gpsimd.tensor_mul(
                    et[:, :, 0:nq], et[:, :, 0:nq], mc_sb[:, :, 0:nq])
                et_ref[(oc, kb)] = et

            def pv_head(g, h):
                """Accumulate P@V for one head over query group g."""
                q0 = g * 512
                oc, hh = h // 2, h % 2
                av = avp.tile([DK + 1, 512], F32, tag="av")
                nc.tensor.matmul(
                    av, zz_sb[0:1, 0:DK + 1], zz_sb[0:1, DK + 1:],
                    start=True, stop=False, skip_group_check=True,
                )
                segs = []
                for kb in range(max(0, 4 * g - 2), 4 * g + 4):
                    a = max(kb * P, q0)
                    b2 = min(kb * P + _nq(kb), q0 + 512)
                    segs.append((kb, a - q0, b2 - q0))
                for i, (kb, a, b2) in enumerate(segs):
                    nc.tensor.matmul(
                        av[:, a:b2],
                        v_sb[:, kb, h * (DK + 1):(h + 1) * (DK + 1)],
                        et_ref[(oc, kb)][:, hh, q0 + a - kb * P:q0 + b2 - kb * P],
                        start=False, stop=(i == len(segs) - 1),
                        skip_group_check=True,
                    )
                # 1/den: evacuate den row, round-trip via DRAM to reshape
                # [1,512] -> [128,4] so the reciprocal is partition-parallel
                den = rp.tile([1, 512], F32, tag="den")
                nc.scalar.activation(
                    out=den, in_=av[DK:DK + 1, :],
                    func=mybir.ActivationFunctionType.Identity, scale=1.0)
                i = g * HG + h
                w1 = nc.sync.dma_start(out=dscr[i:i + 1, :], in_=den)
                tc.chain_iter_dep(f"dw{i}", w1.ins)
                dr = rp.tile([P, 4], F32, tag="dr")
                r1 = nc.sync.dma_start(
                    out=dr, in_=dscr[i:i + 1, :].rearrange(
                        "o (p j) -> (o p) j", p=P))
                tc.chain_iter_dep(f"dw{i}", r1.ins)
                rr = rp.tile([P, 4], F32, tag="rr")
                nc.vector.reciprocal(out=rr, in_=dr)
                w2 = nc.sync.dma_start(
                    out=dscr2[i:i + 1, :].rearrange("o (p j) -> (o p) j", p=P),
                    in_=rr)
                tc.chain_iter_dep(f"db{i}", w2.ins)
                bc = bp.tile([DK, 512], F32, tag="bc")
                r2 = nc.sync.dma_start(
                    out=bc, in_=dscr2[i:i + 1, :].to_broadcast([DK, 512]))
                tc.chain_iter_dep(f"db{i}", r2.ins)
                return av, bc

            def norm_head(g, h, av, bc):
                """Scale by the broadcast 1/den into O^T."""
                q0 = g * 512
                oc, hh = h // 2, h % 2
                nc.vector.tensor_mul(
                    ot_sb[hh * DK:(hh + 1) * DK, oc, q0:q0 + 512],
                    av[0:DK, :], bc)

            def outproj(tb):
                ts = slice(tb * P, (tb + 1) * P)
                ob = op.tile([P, D], BF16, tag="ob")
                for on in range(2):
                    po = fpp.tile([P, 512], F32, tag="po")
                    for kd in range(DG // P):
                        nc.tensor.matmul(
                            po,
                            ot_sb[:, kd, ts],
                            wo_sb[:, kd, on * 512:(on + 1) * 512],
                            start=(kd == 0), stop=(kd == DG // P - 1),
                        )
                    nc.scalar.activation(
                        out=ob[:, on * 512:(on + 1) * 512], in_=po,
                        func=mybir.ActivationFunctionType.Identity, scale=1.0)
                nc.sync.dma_start(out=partial[ts, :], in_=ob)

            for kb in range(0, 4):
                for oc in range(2):
                    st_pair(0, kb, oc)
            for g in range(NG):
                # P@V for this group; 1/den broadcasts lag one head so the
                # PE never waits on the DVE reciprocal
                pend = None
                for h in range(HG):
                    cur = (g, h, *pv_head(g, h))
                    if pend is not None:
                        norm_head(*pend)
                    pend = cur
                norm_head(*pend)
                # interleave next group's scores with this group's
                # out-projection to keep PE fed while Act runs the exps
                seq = []
                if g + 1 < NG:
                    seq = [("st", 4 * (g + 1) + j, oc)
                           for j in range(4) for oc in range(2)]
                ops = [("op", 4 * g + j, None) for j in range(4)]
                merged = []
                si, oi = 0, 0
                order = ["st", "st", "op", "st", "op", "st", "op", "st",
                         "op", "st", "st", "st"]
                for kind in order:
                    if kind == "st" and si < len(seq):
                        merged.append(seq[si]); si += 1
                    elif kind == "op" and oi < len(ops):
                        merged.append(ops[oi]); oi += 1
                merged += seq[si:] + ops[oi:]
                for kind, a, b in merged:
                    if kind == "st":
                        st_pair(g + 1, a, b)
                    else:
                        outproj(a)

            if dbg:
                nc.sync.dma_start(out=d_ht, in_=ht_sb)
                nc.sync.dma_start(out=d_qt, in_=qt_sb)
                nc.sync.dma_start(out=d_kt, in_=kt_sb)
                nc.sync.dma_start(out=d_v, in_=v_sb)
                nc.sync.dma_start(out=d_ot, in_=ot_sb)


def build_nc():
    nc = bacc.Bacc("TRN2", target_bir_lowering=False, debug=False,
                   num_devices=8)
    with tile.TileContext(nc) as tc:
        _body(tc)
    nc.compile()
    return nc


def _prep_core_inputs(x, Wq, Wk, Wv, Wo, gamma, beta):
    """Host-side prep: per-(batch, head-group) input dicts."""
    import ml_dtypes
    BF = ml_dtypes.bfloat16
    B = x.shape[0]
    kk = np.arange(P)[:, None]
    qq = np.arange(P)[None, :]
    md = (kk <= qq).astype(BF)
    mf = (kk > qq).astype(BF)
    m1 = np.concatenate([md, np.ones((P, P), dtype=BF), mf], axis=1)
    mcomb = np.ascontiguousarray(np.stack([m1, m1], axis=1))

    def fold(w):
        # [D, DG] -> [128, KC, DG] with d = c*128 + p
        return np.ascontiguousarray(
            w.reshape(KC, P, DG).transpose(1, 0, 2)).astype(BF)

    in_maps = []
    for b in range(B):
        for g in range(4):
            sl = slice(g * DG, (g + 1) * DG)
            sq = np.float32(1.0 / np.sqrt(DK))
            wq_g = fold(gamma[:, None] * Wq[:, sl] * sq)
            wk_g = fold(gamma[:, None] * Wk[:, sl])
            wv_g = fold(gamma[:, None] * Wv[:, sl])
            wo_g = np.ascontiguousarray(
                Wo[sl, :].reshape(DG // P, P, D).transpose(1, 0, 2)).astype(BF)
            bq_g = ((beta @ Wq[:, sl]) * sq).astype(np.float32)
            bk_g = (beta @ Wk[:, sl]).astype(np.float32)
            bv_g = (beta @ Wv[:, sl]).astype(np.float32)
            in_maps.append({
                "x": np.ascontiguousarray(x[b]).astype(BF),
                "wq": wq_g, "wk": wk_g, "wv": wv_g, "wo": wo_g,
                "bq": np.ascontiguousarray(bq_g.reshape(DG // P, P).T),
                "bk": np.ascontiguousarray(bk_g.reshape(DG // P, P).T),
                "bv": np.tile(bv_g[None, :], (P, 1)),
                "mc": mcomb,
                "zz": np.zeros((1, DK + 1 + 512), dtype=BF),
                "vones": np.ones((P, NT, HG), dtype=BF),
            })
    return in_maps


def _ntff_hook(so_path="/opt/axon/libaxon_pjrt.so"):
    import contextlib
    import ctypes

    lib = ctypes.CDLL(so_path)
    lib.axon_start_nrt_profile.argtypes = [
        ctypes.POINTER(ctypes.c_int64), ctypes.c_size_t]
    lib.axon_start_nrt_profile.restype = ctypes.c_int64
    lib.axon_stop_nrt_profile.argtypes = [ctypes.c_char_p]
    lib.axon_stop_nrt_profile.restype = ctypes.c_int64

    @contextlib.contextmanager
    def _hook(output_dir, device_ids):
        import jax
        jax.devices()
        if device_ids:
            ids = (ctypes.c_int64 * len(device_ids))(*device_ids)
            rc = lib.axon_start_nrt_profile(ids, len(device_ids))
        else:
            rc = lib.axon_start_nrt_profile(None, 0)
        if rc != 0:
            raise RuntimeError(f"axon_start_nrt_profile rc={rc}")
        try:
            yield
        finally:
            n = lib.axon_stop_nrt_profile(str(output_dir).encode())
            print(f"profile: {n} file(s) written to {output_dir}")

    return _hook


def _run_traced(nc, in_maps, trace_dir=None):
    """Execute via PJRT with NTFF capture; return BassKernelResults with
    exec_time_ns and a perfetto trace."""
    import glob
    import tempfile

    import gauge.profiler
    from concourse import bass2jax, bass_utils
    from concourse._compat import FishPath

    neff_dir = trace_dir or tempfile.mkdtemp(prefix="trn_trace_")
    hook = _ntff_hook()
    with hook(neff_dir, [0]):
        results = bass2jax.run_bass_via_pjrt(nc, in_maps, n_cores=len(in_maps))

    ntffs = glob.glob(os.path.join(neff_dir, "*_body*.ntff"))
    if not ntffs:
        print(f"no ntffs in {neff_dir}: {os.listdir(neff_dir)}")
        return bass_utils.BassKernelResults(
            results=results, instructions_and_trace=None,
            profile_json=None, exec_time_ns=None)

    profile = gauge.profiler.Profile(
        profile_path=FishPath(neff_dir),
        kernel_dev_mode=True,
        profile_on_exit=False,
        bass_kernel=nc.m,
        offline_processing=True,
        fname="*_body*",
        metadata={},
    )
    return bass_utils._process_ntff_profile(
        profile, neff_dir, nc, list(range(len(in_maps))),
        None, False, {}, trace_events=False,
    ).as_bass_kernel_results(results)


def kernel(x, Wq, Wk, Wv, Wo, bo, gamma, beta, trace=False):
    global LAST_PROFILE
    x = np.asarray(x, dtype=np.float32)
    Wq, Wk, Wv, Wo = (np.asarray(a, dtype=np.float32) for a in (Wq, Wk, Wv, Wo))
    bo = np.asarray(bo, dtype=np.float32)
    gamma = np.asarray(gamma, dtype=np.float32)
    beta = np.asarray(beta, dtype=np.float32)

    nc = build_nc()
    in_maps = _prep_core_inputs(x, Wq, Wk, Wv, Wo, gamma, beta)
    if trace:
        res = _run_traced(nc, in_maps)
    else:
        res = run_bass_kernel_spmd(nc, in_maps, core_ids=list(range(8)))
    LAST_PROFILE = {"exec_time_ns": res.exec_time_ns}

    B = x.shape[0]
    out = np.empty_like(x)
    for b in range(B):
        acc = x[b] + bo[None, :]
        for g in range(4):
            acc = acc + np.asarray(res.results[b * 4 + g]["partial"],
                                   dtype=np.float32)
        out[b] = acc
    return out


# revision 29
# speedup vs baseline: 1.2127x; 1.0415x over previous
"""Local causal (sliding-window) attention block on 8 TRN2 NeuronCores.

Reference computation (per batch b):
    h = LayerNorm(x) * gamma + beta
    Q = h@Wq, K = h@Wk, V = h@Wv          (heads: 16 x 64)
    S = QK^T/sqrt(dk) masked to causal band of width 256
    out = x + softmax(S)@V @ Wo + bo

Sharding: 8 cores = 2 batches x 4 head-groups (4 heads each).
Each core computes LN(x_b), its head-group's Q/K/V, banded attention,
and a partial out-projection  attn_g @ Wo[g]  (token-major, [T, D]).
Host reduces: out[b] = x[b] + sum_g partial[b,g] + bo.

Implementation notes (v2, overhead-optimized):
- All matmuls run in bf16 (fp32 PSUM accumulation).
- h^T is produced by the DMA XBAR transpose (dma_start(transpose=True)),
  eliminating all PE transposes.
- Attention computes S^T[k, q] tiles directly (k on partitions), so the
  probability tiles feed P@V without any transpose; the softmax
  denominator comes from a ones-column appended to V, and the final
  1/den scaling uses a stride-0 DMA broadcast + one Pool multiply.
- Elementwise work is spread across DVE / Act / Pool to keep the PE the
  only near-saturated engine.
"""

import os

import numpy as np

import concourse.bass as bass
import concourse.tile as tile
from concourse import bacc, mybir
from concourse.bass_utils import run_bass_kernel_spmd

F32 = mybir.dt.float32
F32R = mybir.dt.float32r
BF16 = mybir.dt.bfloat16

T = 2048          # tokens per batch
D = 1024          # model dim
HG = 4            # heads per core
DK = 64           # head dim
DG = HG * DK      # head-group feature width (256)
WIN = 256         # attention window
P = 128           # partitions
NT = T // P       # 16 token tiles
KC = D // P       # 8 feature chunks
NG = NT // 4      # 4 query groups of 512 tokens
LN_EPS = 1e-5
MASKVAL = -1e9

# filled by test.py via run(trace=True)
LAST_PROFILE = {}


def _nq(kb):
    return min(3 * P, (NT - kb) * P)


def _body(tc):
    nc = tc.nc

    x = nc.dram_tensor("x", [T, D], BF16, kind="ExternalInput").ap()
    wq = nc.dram_tensor("wq", [P, KC, DG], BF16, kind="ExternalInput").ap()
    wk = nc.dram_tensor("wk", [P, KC, DG], BF16, kind="ExternalInput").ap()
    wv = nc.dram_tensor("wv", [P, KC, DG], BF16, kind="ExternalInput").ap()
    wo = nc.dram_tensor("wo", [P, DG // P, D], BF16, kind="ExternalInput").ap()
    bq = nc.dram_tensor("bq", [P, DG // P], F32, kind="ExternalInput").ap()
    bk = nc.dram_tensor("bk", [P, DG // P], F32, kind="ExternalInput").ap()
    mc = nc.dram_tensor("mc", [P, HG, 3 * P], BF16, kind="ExternalInput").ap()
    zz = nc.dram_tensor("zz", [1, DK + 1 + 512], BF16, kind="ExternalInput").ap()
    cr = nc.dram_tensor("cr", [1, P + DG], BF16, kind="ExternalInput").ap()
    vones = nc.dram_tensor("vones", [P, NT, HG], BF16, kind="ExternalInput").ap()
    partial = nc.dram_tensor("partial", [T, D], BF16, kind="ExternalOutput").ap()
    dbg = os.environ.get("KDEBUG", "") == "1"
    dscr = nc.dram_tensor("dscr", [NG * 2, 1024], F32,
                          kind="ExternalOutput" if dbg else "Internal").ap()
    dscr2 = nc.dram_tensor("dscr2", [NG * 2, 1024], F32, kind="Internal").ap()
    if dbg:
        d_ht = nc.dram_tensor("d_ht", [P, KC, T], BF16, kind="ExternalOutput").ap()
        d_qt = nc.dram_tensor("d_qt", [P, 2, T], BF16, kind="ExternalOutput").ap()
        d_kt = nc.dram_tensor("d_kt", [P, 2, T], BF16, kind="ExternalOutput").ap()
        d_v = nc.dram_tensor("d_v", [P, NT, HG * (DK + 1)], BF16, kind="ExternalOutput").ap()
        d_ot = nc.dram_tensor("d_ot", [P, 2, T], BF16, kind="ExternalOutput").ap()

    with (
        tc.tile_pool(name="consts", bufs=1) as consts,
        tc.tile_pool(name="big", bufs=1) as big,
    ):
        # ---- resident SBUF tensors ----
        wq_sb = consts.tile([P, KC, DG], BF16, tag="wq")
        wk_sb = consts.tile([P, KC, DG], BF16, tag="wk")
        wv_sb = consts.tile([P, KC, DG], BF16, tag="wv")
        wo_sb = consts.tile([P, DG // P, D], BF16, tag="wo")
        bq_sb = consts.tile([P, DG // P], F32, tag="bq")
        bk_sb = consts.tile([P, DG // P], F32, tag="bk")
        mc_sb = consts.tile([P, HG, 3 * P], BF16, tag="mc")
        zz_sb = consts.tile([1, DK + 1 + 512], BF16, tag="zz")
        cr_sb = consts.tile([1, P + DG], BF16, tag="cr")
        eps_sb = consts.tile([P, 1], F32, tag="eps")

        nc.sync.dma_start(out=wq_sb, in_=wq)
        nc.sync.dma_start(out=wk_sb, in_=wk)
        nc.sync.dma_start(out=wv_sb, in_=wv)
        nc.sync.dma_start(out=wo_sb, in_=wo)
        nc.sync.dma_start(out=bq_sb, in_=bq)
        nc.sync.dma_start(out=bk_sb, in_=bk)
        nc.sync.dma_start(out=mc_sb, in_=mc)
        nc.sync.dma_start(out=zz_sb, in_=zz)
        nc.sync.dma_start(out=cr_sb, in_=cr)
        nc.vector.memset(eps_sb, LN_EPS)

        # h^T (feature-major), Q^T/K^T (feature-major), V (token-major,
        # with a ones column per head for the softmax denominator),
        # O^T (attention output, feature-major)
        ht_sb = big.tile([P, KC, T], BF16, tag="ht")
        qt_sb = big.tile([P, DG // P, T], BF16, tag="qt")
        kt_sb = big.tile([P, DG // P, T], BF16, tag="kt")
        v_sb = big.tile([P, NT, HG * (DK + 1)], BF16, tag="v")
        ot_sb = big.tile([P, DG // P, T], BF16, tag="ot")

        # ones columns of V (denominator trick)
        nc.sync.dma_start(out=v_sb[:, :, DK::DK + 1], in_=vones)

        # ============ Front: LayerNorm + h^T + Q/K/V projections ============
        x_sb = big.tile([P, NT, D], BF16, tag="x")
        for q in range(4):
            nc.sync.dma_start(
                out=x_sb[:, 4 * q:4 * (q + 1), :],
                in_=x.rearrange("(n p) d -> p n d", p=P)[:, 4 * q:4 * (q + 1), :])
        with (
            tc.tile_pool(name="hp", bufs=3) as hp,
            tc.tile_pool(name="lnst", bufs=4) as lnst,
            tc.tile_pool(name="qkp", bufs=2, space="PSUM") as qkp,
            tc.tile_pool(name="vp", bufs=2, space="PSUM") as vp,
        ):
            for tb in range(NT):
                ts = slice(tb * P, (tb + 1) * P)
                xt = x_sb[:, tb, :]

                stats = lnst.tile([P, 2, 6], F32, tag="stats")
                xg = xt.rearrange("p (g d) -> p g d", g=2)
                nc.vector.bn_stats(out=stats[:, 0, :], in_=xg[:, 0, :])
                nc.vector.bn_stats(out=stats[:, 1, :], in_=xg[:, 1, :])
                mv = lnst.tile([P, 2], F32, tag="mv")
                nc.vector.bn_aggr(out=mv, in_=stats)

                rstd = lnst.tile([P, 1], F32, tag="rstd")
                nc.scalar.activation(
                    out=rstd, in_=mv[:, 1:2],
                    func=mybir.ActivationFunctionType.Sqrt,
                    bias=eps_sb, scale=1.0,
                )
                nc.vector.reciprocal(out=rstd, in_=rstd)
                nmr = lnst.tile([P, 1], F32, tag="nmr")
                nc.vector.tensor_scalar(
                    out=nmr, in0=mv[:, 0:1], scalar1=rstd, scalar2=-1.0,
                    op0=mybir.AluOpType.mult, op1=mybir.AluOpType.mult,
                )

                hn = hp.tile([P, D], BF16, tag="hn")
                nc.gpsimd.tensor_scalar(
                    out=hn, in0=xt, scalar1=rstd, scalar2=nmr,
                    op0=mybir.AluOpType.mult, op1=mybir.AluOpType.add,
                )
                # h^T via DMA XBAR transpose: ht[p, c, t] = hn[t, c*128+p]
                nc.sync.dma_start(out=ht_sb[:, :, ts], in_=hn, transpose=True)

                # V projection for this token tile (token-major);
                # bv enters as a rank-1 (ones x bv) accumulation
                ps = vp.tile([P, DG], F32, tag="psv")
                for kc in range(KC):
                    nc.tensor.matmul(
                        ps, ht_sb[:, kc, ts], wv_sb[:, kc, :],
                        start=(kc == 0), stop=False,
                    )
                nc.tensor.matmul(
                    ps, cr_sb[0:1, 0:P], cr_sb[0:1, P:],
                    start=False, stop=True,
                )
                nc.scalar.activation(
                    out=v_sb[:, tb, :].rearrange(
                        "p (h d) -> p h d", d=DK + 1)[:, :, 0:DK],
                    in_=ps.rearrange("p (h d) -> p h d", d=DK),
                    func=mybir.ActivationFunctionType.Identity, scale=1.0)

                # Q^T / K^T per completed 512-token slice
                if tb % 4 == 3:
                    sl = tb // 4
                    ss = slice(sl * 512, (sl + 1) * 512)
                    for w_sb, b_sb, dst in ((wq_sb, bq_sb, qt_sb),
                                            (wk_sb, bk_sb, kt_sb)):
                        for oc in range(DG // P):
                            pq = qkp.tile([P, 512], F32, tag="psqk")
                            for kc in range(KC):
                                nc.tensor.matmul(
                                    pq,
                                    w_sb[:, kc, oc * P:(oc + 1) * P],
                                    ht_sb[:, kc, ss],
                                    start=(kc == 0), stop=(kc == KC - 1),
                                )
                            nc.vector.tensor_scalar_add(
                                dst[:, oc, ss], pq, b_sb[:, oc:oc + 1])

        # ============ Attention (S^T formulation) + out-projection ============
        with (
            tc.tile_pool(name="sp", bufs=2, space="PSUM") as sp,
            tc.tile_pool(name="avp", bufs=2, space="PSUM") as avp,
            tc.tile_pool(name="fpp", bufs=2, space="PSUM") as fpp,
            tc.tile_pool(name="ep", bufs=10) as ep,
            tc.tile_pool(name="rp", bufs=4) as rp,
            tc.tile_pool(name="bp", bufs=3) as bp,
            tc.tile_pool(name="op", bufs=3) as op,
        ):
            et_ref = {}

            def st_quad(kb):
                """S^T + exp + band mask for all 4 heads of key block kb."""
                nq = _nq(kb)
                ks = slice(kb * P, (kb + 1) * P)
                et = ep.tile([P, HG, 3 * P], BF16, tag="et")
                for j in range(2):
                    s2 = sp.tile([P, 2, 512], F32, tag="s2")
                    for hh in range(2):
                        h = 2 * j + hh
                        p0 = (h % 2) * DK
                        nc.tensor.matmul(
                            s2[:, hh, 0:nq],
                            kt_sb[p0:p0 + DK, h // 2, ks],
                            qt_sb[p0:p0 + DK, h // 2, kb * P:kb * P + nq],
                            start=True, stop=True,
                        )
                    nc.scalar.activation(
                        out=et[:, 2 * j:2 * j + 2, 0:nq], in_=s2[:, :, 0:nq],
                        func=mybir.ActivationFunctionType.Exp,
                    )
                nc.gpsimd.tensor_mul(
                    et[:, :, 0:nq], et[:, :, 0:nq], mc_sb[:, :, 0:nq])
                et_ref[kb] = et

            def pv_head(g, h, den2, j):
                """P@V for one head; den row lands in den2[j] (own tile)."""
                q0 = g * 512
                oc, hh = h // 2, h % 2
                av = avp.tile([DK + 1, 512], F32, tag="av")
                nc.tensor.matmul(
                    av, zz_sb[0:1, 0:DK + 1], zz_sb[0:1, DK + 1:],
                    start=True, stop=False, skip_group_check=True,
                )
                segs = []
                for kb in range(max(0, 4 * g - 2), 4 * g + 4):
                    a = max(kb * P, q0)
                    b2 = min(kb * P + _nq(kb), q0 + 512)
                    segs.append((kb, a - q0, b2 - q0))
                for i, (kb, a, b2) in enumerate(segs):
                    nc.tensor.matmul(
                        av[:, a:b2],
                        v_sb[:, kb, h * (DK + 1):(h + 1) * (DK + 1)],
                        et_ref[kb][:, h, q0 + a - kb * P:q0 + b2 - kb * P],
                        start=False, stop=(i == len(segs) - 1),
                        skip_group_check=True,
                    )
                nc.scalar.activation(
                    out=den2[j], in_=av[DK:DK + 1, :],
                    func=mybir.ActivationFunctionType.Identity, scale=1.0)
                return av

            def recip_pair(g, pair):
                """1/den for two heads: round-trip via DRAM so the
                reciprocal runs partition-parallel ([1,1024]->[128,8])."""
                i = g * 2 + pair
                den2 = recip_pair.den2
                for j in range(2):
                    w1 = nc.sync.dma_start(
                        out=dscr[i:i + 1, j * 512:(j + 1) * 512], in_=den2[j])
                    tc.chain_iter_dep(f"dw{i}", w1.ins)
                dr = rp.tile([P, 8], F32, tag="dr")
                r1 = nc.sync.dma_start(
                    out=dr, in_=dscr[i:i + 1, :].rearrange(
                        "o (p j) -> (o p) j", p=P))
                tc.chain_iter_dep(f"dw{i}", r1.ins)
                rr = rp.tile([P, 8], F32, tag="rr")
                nc.vector.reciprocal(out=rr, in_=dr)
                w2 = nc.sync.dma_start(
                    out=dscr2[i:i + 1, :].rearrange("o (p j) -> (o p) j", p=P),
                    in_=rr)
                tc.chain_iter_dep(f"db{i}", w2.ins)
                bc = bp.tile([DK, 2, 512], F32, tag="bc")
                r2 = nc.sync.dma_start(
                    out=bc, in_=dscr2[i:i + 1, :].rearrange(
                        "o (j q) -> o j q", j=2).to_broadcast([DK, 2, 512]))
                tc.chain_iter_dep(f"db{i}", r2.ins)
                return bc

            def norm_head(g, h, av, bc, j):
                q0 = g * 512
                oc, hh = h // 2, h % 2
                nc.vector.tensor_mul(
                    ot_sb[hh * DK:(hh + 1) * DK, oc, q0:q0 + 512],
                    av[0:DK, :], bc[:, j, :])

            def outproj(tb):
                ts = slice(tb * P, (tb + 1) * P)
                ob = op.tile([P, D], BF16, tag="ob")
                for on in range(2):
                    po = fpp.tile([P, 512], F32, tag="po")
                    for kd in range(DG // P):
                        nc.tensor.matmul(
                            po,
                            ot_sb[:, kd, ts],
                            wo_sb[:, kd, on * 512:(on + 1) * 512],
                            start=(kd == 0), stop=(kd == DG // P - 1),
                        )
                    nc.scalar.activation(
                        out=ob[:, on * 512:(on + 1) * 512], in_=po,
                        func=mybir.ActivationFunctionType.Identity, scale=1.0)
                nc.sync.dma_start(out=partial[ts, :], in_=ob)

            for kb in range(0, 4):
                st_quad(kb)
            for g in range(NG):
                nxt = [4 * (g + 1) + j for j in range(4)] if g + 1 < NG else []
                # heads 0,1 -> pair reciprocal -> normalize; ST quads of the
                # next group are threaded between to keep the PE busy while
                # the 1/den DMA round-trip is in flight
                dena0 = rp.tile([1, 512], F32, tag="den2a")
                dena1 = rp.tile([1, 512], F32, tag="den2b")
                den2a = [dena0, dena1]
                recip_pair.den2 = den2a
                av0 = pv_head(g, 0, den2a, 0)
                av1 = pv_head(g, 1, den2a, 1)
                if nxt:
                    st_quad(nxt[0])
                bca = recip_pair(g, 0)
                if nxt:
                    st_quad(nxt[1])
                norm_head(g, 0, av0, bca, 0)
                norm_head(g, 1, av1, bca, 1)
                denb0 = rp.tile([1, 512], F32, tag="den2a")
                denb1 = rp.tile([1, 512], F32, tag="den2b")
                den2b = [denb0, denb1]
                recip_pair.den2 = den2b
                av2 = pv_head(g, 2, den2b, 0)
                av3 = pv_head(g, 3, den2b, 1)
                if nxt:
                    st_quad(nxt[2])
                bcb = recip_pair(g, 1)
                if nxt:
                    st_quad(nxt[3])
                norm_head(g, 2, av2, bcb, 0)
                norm_head(g, 3, av3, bcb, 1)
                for tb in range(4 * g, 4 * g + 4):
                    outproj(tb)

            if dbg:
                nc.sync.dma_start(out=d_ht, in_=ht_sb)
                nc.sync.dma_start(out=d_qt, in_=qt_sb)
                nc.sync.dma_start(out=d_kt, in_=kt_sb)
                nc.sync.dma_start(out=d_v, in_=v_sb)
                nc.sync.dma_start(out=d_ot, in_=ot_sb)


def build_nc():
    nc = bacc.Bacc("TRN2", target_bir_lowering=False, debug=False,
                   num_devices=8)
    with tile.TileContext(nc) as tc:
        _body(tc)
    nc.compile()
    return nc


def _prep_core_inputs(x, Wq, Wk, Wv, Wo, gamma, beta):
    """Host-side prep: per-(batch, head-group) input dicts."""
    import ml_dtypes
    BF = ml_dtypes.bfloat16
    B = x.shape[0]
    kk = np.arange(P)[:, None]
    qq = np.arange(P)[None, :]
    md = (kk <= qq).astype(BF)
    mf = (kk > qq).astype(BF)
    m1 = np.concatenate([md, np.ones((P, P), dtype=BF), mf], axis=1)
    mcomb = np.ascontiguousarray(np.stack([m1] * 4, axis=1))

    def fold(w):
        # [D, DG] -> [128, KC, DG] with d = c*128 + p
        return np.ascontiguousarray(
            w.reshape(KC, P, DG).transpose(1, 0, 2)).astype(BF)

    in_maps = []
    for b in range(B):
        for g in range(4):
            sl = slice(g * DG, (g + 1) * DG)
            sq = np.float32(1.0 / np.sqrt(DK))
            wq_g = fold(gamma[:, None] * Wq[:, sl] * sq)
            wk_g = fold(gamma[:, None] * Wk[:, sl])
            wv_g = fold(gamma[:, None] * Wv[:, sl])
            wo_g = np.ascontiguousarray(
                Wo[sl, :].reshape(DG // P, P, D).transpose(1, 0, 2)).astype(BF)
            bq_g = ((beta @ Wq[:, sl]) * sq).astype(np.float32)
            bk_g = (beta @ Wk[:, sl]).astype(np.float32)
            bv_g = (beta @ Wv[:, sl]).astype(np.float32)
            crow = np.concatenate(
                [np.ones(P, dtype=np.float32), bv_g]).astype(BF)
            in_maps.append({
                "x": np.ascontiguousarray(x[b]).astype(BF),
                "wq": wq_g, "wk": wk_g, "wv": wv_g, "wo": wo_g,
                "bq": np.ascontiguousarray(bq_g.reshape(DG // P, P).T),
                "bk": np.ascontiguousarray(bk_g.reshape(DG // P, P).T),
                "cr": crow[None, :],
                "mc": mcomb,
                "zz": np.zeros((1, DK + 1 + 512), dtype=BF),
                "vones": np.ones((P, NT, HG), dtype=BF),
            })
    return in_maps


def _ntff_hook(so_path="/opt/axon/libaxon_pjrt.so"):
    import contextlib
    import ctypes

    lib = ctypes.CDLL(so_path)
    lib.axon_start_nrt_profile.argtypes = [
        ctypes.POINTER(ctypes.c_int64), ctypes.c_size_t]
    lib.axon_start_nrt_profile.restype = ctypes.c_int64
    lib.axon_stop_nrt_profile.argtypes = [ctypes.c_char_p]
    lib.axon_stop_nrt_profile.restype = ctypes.c_int64

    @contextlib.contextmanager
    def _hook(output_dir, device_ids):
        import jax
        jax.devices()
        if device_ids:
            ids = (ctypes.c_int64 * len(device_ids))(*device_ids)
            rc = lib.axon_start_nrt_profile(ids, len(device_ids))
        else:
            rc = lib.axon_start_nrt_profile(None, 0)
        if rc != 0:
            raise RuntimeError(f"axon_start_nrt_profile rc={rc}")
        try:
            yield
        finally:
            n = lib.axon_stop_nrt_profile(str(output_dir).encode())
            print(f"profile: {n} file(s) written to {output_dir}")

    return _hook


def _run_traced(nc, in_maps, trace_dir=None):
    """Execute via PJRT with NTFF capture; return BassKernelResults with
    exec_time_ns and a perfetto trace."""
    import glob
    import tempfile

    import gauge.profiler
    from concourse import bass2jax, bass_utils
    from concourse._compat import FishPath

    neff_dir = trace_dir or tempfile.mkdtemp(prefix="trn_trace_")
    hook = _ntff_hook()
    with hook(neff_dir, [0]):
        results = bass2jax.run_bass_via_pjrt(nc, in_maps, n_cores=len(in_maps))

    ntffs = glob.glob(os.path.join(neff_dir, "*_body*.ntff"))
    if not ntffs:
        print(f"no ntffs in {neff_dir}: {os.listdir(neff_dir)}")
        return bass_utils.BassKernelResults(
            results=results, instructions_and_trace=None,
            profile_json=None, exec_time_ns=None)

    profile = gauge.profiler.Profile(
        profile_path=FishPath(neff_dir),
        kernel_dev_mode=True,
        profile_on_exit=False,
        bass_kernel=nc.m,
        offline_processing=True,
        fname="*_body*",
        metadata={},
    )
    return bass_utils._process_ntff_profile(
        profile, neff_dir, nc, list(range(len(in_maps))),
        None, False, {}, trace_events=False,
    ).as_bass_kernel_results(results)


def kernel(x, Wq, Wk, Wv, Wo, bo, gamma, beta, trace=False):
    global LAST_PROFILE
    x = np.asarray(x, dtype=np.float32)
    Wq, Wk, Wv, Wo = (np.asarray(a, dtype=np.float32) for a in (Wq, Wk, Wv, Wo))
    bo = np.asarray(bo, dtype=np.float32)
    gamma = np.asarray(gamma, dtype=np.float32)
    beta = np.asarray(beta, dtype=np.float32)

    nc = build_nc()
    in_maps = _prep_core_inputs(x, Wq, Wk, Wv, Wo, gamma, beta)
    if trace:
        res = _run_traced(nc, in_maps)
    else:
        res = run_bass_kernel_spmd(nc, in_maps, core_ids=list(range(8)))
    LAST_PROFILE = {"exec_time_ns": res.exec_time_ns}

    B = x.shape[0]
    out = np.empty_like(x)
    for b in range(B):
        acc = x[b] + bo[None, :]
        for g in range(4):
            acc = acc + np.asarray(res.results[b * 4 + g]["partial"],
                                   dtype=np.float32)
        out[b] = acc
    return out
